# revision 6
# baseline (speedup 1.0000x reference)
"""Trainium2 Bass kernel for a 16-head causal MultiHeadAttention layer.

Shapes (hardcoded): x [4, 1024, 1024] f32; Wq/Wk/Wv [16, 1024, 64]; Wo [1024, 1024]; bo [1024].
Sharding over 8 cores: core i handles (batch b = i//2, head-group g = i%2 covering 8 heads).
Each core computes its batch's attention for its 8 heads plus the partial output
projection against Wo's matching input-dim slice; host sums the two partials per
batch and adds the bias.

Device algorithm (per core), all matmuls in float32r:
  1. xT = x.T via PE transposes                      [c, t]
  2. QT/KT = W.T @ xT (heads packed in pairs)        [s(2 heads), t]
     V = x @ Wv (all 8 heads)                        [u, h*64+s]
  3. per head: scoresT[u, t] = K @ QT (only causal-valid tiles), exp on ACT
     (scale 1/8 folded in), diagonal tiles masked multiplicatively
  4. per head pair: AV matmul with lhsT = [V_h0 | ones | V_h1] slices so one
     psum holds AVT rows and replicated row-sums; DVE reciprocal + multiply
     normalizes into concatT [hs, t]
  5. out[t, d] = concatT.T @ WoT, DMA to DRAM
"""

import os
import sys
from contextlib import ExitStack

import numpy as np

for _p in ("/opt/trn_rl_repo", "/opt/pypackages"):
    if _p not in sys.path:
        sys.path.append(_p)

import concourse.bass as bass
import concourse.mybir as mybir
import concourse.tile as tile
from concourse import bacc, bass_utils
from concourse.masks import make_identity

F32 = mybir.dt.float32
F32R = mybir.dt.float32r

B, T, D, HS = 4, 1024, 1024, 64
NH = 8          # heads per core
NPAIR = 4       # head pairs per core
P = 128
KT_TILES = 8    # c tiles of 128
TB = 8          # t blocks of 128
SCALE = 1.0 / 8.0

# normalization strategy: "shifted_recip" uses a partition-shifted 1-src
# reciprocal; "split" uses extra M=64 matmuls and no partition shifts at all.
NORM_MODE = os.environ.get("MHA_NORM_MODE", "shifted_recip")


def r(ap):
    return ap


def emit(tc: tile.TileContext, io: dict):
    nc = tc.nc
    x_d, wq_d, wk_d, wv_d, wo_d, mask_d, y_d = (
        io["x"], io["wq"], io["wk"], io["wv"], io["wo"], io["mask"], io["y"],
    )

    with ExitStack() as outer:
        const = outer.enter_context(tc.tile_pool(name="const", bufs=1))
        identity = const.tile([P, P], F32, tag="identity")
        make_identity(nc, identity[:])
        mask_sb = const.tile([P, P], F32R, tag="mask")
        nc.sync.dma_start(mask_sb[:], mask_d[:])
        wo_sb = const.tile([P, 4, D], F32R, tag="wo")
        nc.sync.dma_start(wo_sb[:], wo_d[:])

        persist = outer.enter_context(tc.tile_pool(name="persist", bufs=1))
        qt_sb = persist.tile([P, NPAIR, T], F32R, tag="qt")
        kt_sb = persist.tile([P, NPAIR, T], F32R, tag="kt")
        # per (u-block, pair): [V_h0 (0:64) | ones (64:128) | V_h1 (128:192)]
        vpad_sb = persist.tile([P, TB, NPAIR, 192], F32R, tag="vpad")
        ones_f32 = persist.tile([P, 64], F32, tag="ones_f32")
        nc.vector.memset(ones_f32[:], 1.0)
        for _ub in range(TB):
            for _p in range(NPAIR):
                nc.vector.tensor_copy(vpad_sb[:, _ub, _p, 64:128], ones_f32[:])
        concat_sb = persist.tile([P, NPAIR, T], F32R, tag="concat")

        # ---- phase 1+2: xT, then Q/K/V projections ----
        with ExitStack() as ph:
            xpool = ph.enter_context(tc.tile_pool(name="xpool", bufs=3))
            xtpool = ph.enter_context(tc.tile_pool(name="xtpool", bufs=1))
            wpool = ph.enter_context(tc.tile_pool(name="wpool", bufs=1))
            pst = ph.enter_context(tc.tile_pool(name="pst", bufs=2, space="PSUM"))
            psqk = ph.enter_context(tc.tile_pool(name="psqk", bufs=2, space="PSUM"))
            psv = ph.enter_context(tc.tile_pool(name="psv", bufs=2, space="PSUM"))

            wq_sb = wpool.tile([P, KT_TILES, 512], F32R, tag="wq")
            wk_sb = wpool.tile([P, KT_TILES, 512], F32R, tag="wk")
            wv_sb = wpool.tile([P, KT_TILES, 512], F32R, tag="wv")
            nc.sync.dma_start(wq_sb[:], wq_d[:])
            nc.sync.dma_start(wk_sb[:], wk_d[:])
            nc.sync.dma_start(wv_sb[:], wv_d[:])

            xt_sb = xtpool.tile([P, KT_TILES, T], F32R, tag="xt")
            for tt in range(TB):
                x_tile = xpool.tile([P, D], F32, tag="x")
                nc.sync.dma_start(x_tile[:], x_d[tt * P:(tt + 1) * P, :])
                for ct in range(KT_TILES):
                    ps = pst.tile([P, P], F32, tag="pst")
                    nc.tensor.transpose(ps[:], x_tile[:, ct * P:(ct + 1) * P], identity[:])
                    nc.vector.tensor_copy(xt_sb[:, ct, tt * P:(tt + 1) * P], ps[:])

            # Q^T, K^T: per pair, per t-half; lhsT = W slice, rhs = xT
            for p in range(NPAIR):
                for th in range(2):
                    psq = psqk.tile([P, 512], F32, tag="psq")
                    psk = psqk.tile([P, 512], F32, tag="psk")
                    for ct in range(KT_TILES):
                        nc.tensor.matmul(
                            psq[:], r(wq_sb[:, ct, p * P:(p + 1) * P]),
                            r(xt_sb[:, ct, th * 512:(th + 1) * 512]),
                            start=(ct == 0), stop=(ct == KT_TILES - 1))
                        nc.tensor.matmul(
                            psk[:], r(wk_sb[:, ct, p * P:(p + 1) * P]),
                            r(xt_sb[:, ct, th * 512:(th + 1) * 512]),
                            start=(ct == 0), stop=(ct == KT_TILES - 1))
                    nc.vector.tensor_copy(qt_sb[:, p, th * 512:(th + 1) * 512], psq[:])
                    nc.vector.tensor_copy(kt_sb[:, p, th * 512:(th + 1) * 512], psk[:])

            # V: per u-block, all 8 heads at once; lhsT = xT slice, rhs = Wv
            for ub in range(TB):
                pv = psv.tile([P, 512], F32, tag="psv")
                for ct in range(KT_TILES):
                    nc.tensor.matmul(
                        pv[:], r(xt_sb[:, ct, ub * P:(ub + 1) * P]),
                        r(wv_sb[:, ct, :]),
                        start=(ct == 0), stop=(ct == KT_TILES - 1))
                for p in range(NPAIR):
                    nc.vector.tensor_copy(vpad_sb[:, ub, p, 0:64], pv[:, p * P:p * P + 64])
                    nc.vector.tensor_copy(vpad_sb[:, ub, p, 128:192], pv[:, p * P + 64:(p + 1) * P])

        # ---- phase 3+4: attention per pair ----
        with ExitStack() as ph:
            et_pool = ph.enter_context(tc.tile_pool(name="expT", bufs=1))
            rec_pool = ph.enter_context(tc.tile_pool(name="rec", bufs=2))
            pss = ph.enter_context(tc.tile_pool(name="pss", bufs=2, space="PSUM"))
            psav = ph.enter_context(tc.tile_pool(name="psav", bufs=2, space="PSUM"))

            for p in range(NPAIR):
                et = [et_pool.tile([P, TB, T], F32R, tag=f"expT{hh}", name=f"expT{hh}")
                      for hh in range(2)]
                for hh in range(2):
                    hsl = slice(hh * 64, hh * 64 + 64)
                    for ub in range(TB):
                        tv = ub * P
                        ps_s = pss.tile([P, T], F32, tag="pss")
                        for th in range(2):
                            lo, hi = th * 512, (th + 1) * 512
                            if hi <= tv:
                                continue  # fully masked
                            c0 = max(lo, tv)
                            nc.tensor.matmul(
                                ps_s[:, c0:hi],
                                r(kt_sb[hsl, p, tv:tv + P]),
                                r(qt_sb[hsl, p, c0:hi]),
                                start=True, stop=True)
                        nc.scalar.activation(
                            et[hh][:, ub, tv:T], ps_s[:, tv:T],
                            mybir.ActivationFunctionType.Exp, scale=SCALE)
                        # causal mask on the diagonal block
                        nc.vector.tensor_mul(
                            et[hh][:, ub, tv:tv + P], et[hh][:, ub, tv:tv + P], mask_sb[:])

                for th in range(2):
                    lo, hi = th * 512, (th + 1) * 512
                    ps_a = psav.tile([P, 512], F32, tag="psa")
                    ps_b = psav.tile([P, 512], F32, tag="psb")
                    ubs = [ub for ub in range(TB) if ub * P < hi]
                    for i, ub in enumerate(ubs):
                        tv = ub * P
                        c0 = max(lo, tv) - lo
                        nc.tensor.matmul(
                            ps_a[:, c0:512], r(vpad_sb[:, ub, p, 0:128]),
                            r(et[0][:, ub, lo + c0:hi]),
                            start=(i == 0), stop=(i == len(ubs) - 1))
                        nc.tensor.matmul(
                            ps_b[:, c0:512], r(vpad_sb[:, ub, p, 64:192]),
                            r(et[1][:, ub, lo + c0:hi]),
                            start=(i == 0), stop=(i == len(ubs) - 1))
                    # ps_a = [AVT_h0 ; rowsum_h0 replicated]
                    # ps_b = [rowsum_h1 replicated ; AVT_h1]
                    if NORM_MODE == "shifted_recip":
                        rec = rec_pool.tile([P, 512], F32, tag="rec")
                        nc.vector.reciprocal(rec[0:64, :], ps_a[64:128, :])
                        nc.vector.reciprocal(rec[64:128, :], ps_b[0:64, :])
                        nc.vector.tensor_mul(
                            concat_sb[0:64, p, lo:hi], ps_a[0:64, :], rec[0:64, :])
                        nc.vector.tensor_mul(
                            concat_sb[64:128, p, lo:hi], ps_b[64:128, :], rec[64:128, :])
                    else:  # "aligned": reciprocal in place, then shifted copy
                        rec = rec_pool.tile([P, 512], F32, tag="rec")
                        nc.vector.reciprocal(rec[64:128, :], ps_a[64:128, :])
                        nc.vector.reciprocal(rec[0:64, :], ps_b[0:64, :])
                        rec2 = rec_pool.tile([P, 512], F32, tag="rec2")
                        nc.vector.tensor_copy(rec2[0:64, :], rec[64:128, :])
                        nc.vector.tensor_copy(rec2[64:128, :], rec[0:64, :])
                        nc.vector.tensor_mul(
                            concat_sb[0:64, p, lo:hi], ps_a[0:64, :], rec2[0:64, :])
                        nc.vector.tensor_mul(
                            concat_sb[64:128, p, lo:hi], ps_b[64:128, :], rec2[64:128, :])

        # ---- phase 5: output projection ----
        with ExitStack() as ph:
            opool = ph.enter_context(tc.tile_pool(name="out", bufs=3))
            psp = ph.enter_context(tc.tile_pool(name="psp", bufs=2, space="PSUM"))
            for tb in range(TB):
                o_tile = opool.tile([P, D], F32, tag="o")
                for dh in range(2):
                    pp = psp.tile([P, 512], F32, tag="psp")
                    for kp in range(NPAIR):
                        nc.tensor.matmul(
                            pp[:], r(concat_sb[:, kp, tb * P:(tb + 1) * P]),
                            r(wo_sb[:, kp, dh * 512:(dh + 1) * 512]),
                            start=(kp == 0), stop=(kp == NPAIR - 1))
                    nc.vector.tensor_copy(o_tile[:, dh * 512:(dh + 1) * 512], pp[:])
                nc.sync.dma_start(y_d[tb * P:(tb + 1) * P, :], o_tile[:])


def build():
    nc = bacc.Bacc(
        "TRN2", target_bir_lowering=False, debug=False,
        enable_asserts=False, num_devices=8,
    )
    io = {
        "x": nc.dram_tensor("x", [T, D], F32, kind="ExternalInput").ap(),
        "wq": nc.dram_tensor("wq", [P, KT_TILES, 512], F32R, kind="ExternalInput").ap(),
        "wk": nc.dram_tensor("wk", [P, KT_TILES, 512], F32R, kind="ExternalInput").ap(),
        "wv": nc.dram_tensor("wv", [P, KT_TILES, 512], F32R, kind="ExternalInput").ap(),
        "wo": nc.dram_tensor("wo", [P, 4, D], F32R, kind="ExternalInput").ap(),
        "mask": nc.dram_tensor("mask", [P, P], F32R, kind="ExternalInput").ap(),
        "y": nc.dram_tensor("y", [T, D], F32, kind="ExternalOutput").ap(),
    }
    with tile.TileContext(nc) as tc:
        emit(tc, io)
    nc.compile()
    return nc


def _prep_w_qkv(w):  # [8, 1024, 64] -> [128, 8, 512]
    arr = np.ascontiguousarray(w).transpose(1, 0, 2).reshape(D, 512)
    return np.ascontiguousarray(arr.reshape(8, P, 512).transpose(1, 0, 2))


def _prep_wo(wo, g):  # [1024, 1024] -> [128, 4, 1024] slice for group g
    wot = np.ascontiguousarray(wo[:, g * 512:(g + 1) * 512].T)  # [hs, d]
    return np.ascontiguousarray(wot.reshape(4, P, D).transpose(1, 0, 2))


def make_in_maps(x, Wq, Wk, Wv, Wo, bo):
    mask = (np.arange(P)[None, :] >= np.arange(P)[:, None]).astype(np.float32)
    in_maps = []
    for core in range(8):
        b, g = core // 2, core % 2
        hs = slice(g * NH, (g + 1) * NH)
        in_maps.append({
            "x": np.ascontiguousarray(x[b], np.float32),
            "wq": _prep_w_qkv(Wq[hs]).astype(np.float32),
            "wk": _prep_w_qkv(Wk[hs]).astype(np.float32),
            "wv": _prep_w_qkv(Wv[hs]).astype(np.float32),
            "wo": _prep_wo(Wo, g).astype(np.float32),
            "mask": mask,
        })
    return in_maps


_CACHE = {}


def kernel(x, Wq, Wk, Wv, Wo, bo):
    if "nc" not in _CACHE:
        _CACHE["nc"] = build()
    nc = _CACHE["nc"]
    in_maps = make_in_maps(x, Wq, Wk, Wv, Wo, bo)
    res = bass_utils.run_bass_kernel_spmd(nc, in_maps, core_ids=list(range(8)))
    parts = [res.results[i]["y"] for i in range(8)]
    out = np.stack(
        [parts[2 * b] + parts[2 * b + 1] + bo[None, :].astype(np.float32)
         for b in range(B)]
    ).astype(np.float32)
    return out


if __name__ == "__main__":
    nc = build()
    print("built ok; instructions:", sum(1 for _ in nc.m.functions[0].instructions)
          if hasattr(nc.m.functions[0], "instructions") else "?")
